# revision 18
# baseline (speedup 1.0000x reference)
"""Causal self-attention with RoPE (B=2, T=2048, C=2048, H=16, D=128) on 8 TRN2 cores.

Sharding: tensor-parallel over heads (2 heads per core).
  - column-parallel fused QKV projection (each core computes q,k,v for its 2 heads)
  - RoPE + causal flash-style attention per (batch, head) on-core
  - AllToAll to regroup the attention output from head-sharded to token-sharded
    (w_proj streams into SBUF concurrently with the collective)
  - token-parallel output projection (each core produces 512 token rows of y)

Layouts (per core):
  xT      (2048 c, 4096 tok)  f32r   x transposed, replicated
  wqk     (16, 128, 512)      f32r   [c-chunk, c, q_h0|q_h1|k_h0|k_h1]
  wv      (16, 128, 256)      f32r   [c-chunk, c, v_h0|v_h1]
  wproj   (16, 128, 2048)     f32r   w_proj.T chunked, replicated
  cosT    (128, 2048)         f32    RoPE cos, (D, T)
  sinTs   (128, 2048)         f32    RoPE sin, (D, T), rows 0:64 negated
  masksd  (128, 4, 512)       bf16   causal 0/1 masks for diagonal k-tiles
  y_out   (512, 2048)         f32    output rows for this core's token slice
"""

import contextlib

import numpy as np
import ml_dtypes

import concourse.bass as bass
import concourse.bacc as bacc
import concourse.mybir as mybir
import concourse.tile as tile
from concourse import masks as cmasks
from concourse.bass_utils import run_bass_kernel_spmd

N_CORES = 8
B, T, C = 2, 2048, 2048
H, D = 16, 128
H_LOC = H // N_CORES          # 2 heads per core
BT = B * T                    # 4096 tokens
TOK_PC = BT // N_CORES        # 512 tokens per core (proj phase)
SCALE = 1.0 / float(np.sqrt(D))
ROPE_BASE = 10000.0

F32 = mybir.dt.float32
F32R = mybir.dt.float32r
BF16 = mybir.dt.bfloat16

N_GRP = BT // 512             # 8 token groups of 512 in QKV phase
N_CCH = C // 128              # 16 contraction chunks


def build(repeat=None, use_collective=True, phases=(1, 2, 3)):
    """Build the SPMD Bass program. repeat=R wraps compute in a For_i timing
    loop (collective replaced by local DRAM bounce)."""
    nc = bacc.Bacc("TRN2", target_bir_lowering=False, debug=False,
                   num_devices=N_CORES)

    xT_d = nc.dram_tensor("xT", [C, BT], F32R, kind="ExternalInput").ap()
    wqk_d = nc.dram_tensor("wqk", [N_CCH, 128, 512], F32R, kind="ExternalInput").ap()
    wv_d = nc.dram_tensor("wv", [N_CCH, 128, 256], F32R, kind="ExternalInput").ap()
    wproj_d = nc.dram_tensor("wproj", [N_CCH, 128, C], F32R, kind="ExternalInput").ap()
    cosT_d = nc.dram_tensor("cosT", [128, T], F32, kind="ExternalInput").ap()
    sinTs_d = nc.dram_tensor("sinTs", [128, T], F32, kind="ExternalInput").ap()
    masksd_d = nc.dram_tensor("masksd", [128, 4, 512], BF16, kind="ExternalInput").ap()
    y_d = nc.dram_tensor("y", [TOK_PC, C], F32, kind="ExternalOutput").ap()

    a2a_in = [nc.dram_tensor(f"a2a_in{i}", [N_CORES, 256, 256], F32R).ap()
              for i in range(2)]
    a2a_out = [nc.dram_tensor(f"a2a_out{i}", [N_CORES, 256, 256], F32R).ap()
               for i in range(2)]

    with tile.TileContext(nc) as tc:
        _emit(nc, tc, locals(), repeat, use_collective, phases)
    nc.compile()
    return nc


def _emit(nc, tc, t_, repeat, use_collective, phases=(1, 2, 3)):
    xT_d, wqk_d, wv_d, wproj_d = t_["xT_d"], t_["wqk_d"], t_["wv_d"], t_["wproj_d"]
    cosT_d, sinTs_d, masksd_d, y_d = t_["cosT_d"], t_["sinTs_d"], t_["masksd_d"], t_["y_d"]
    a2a_in, a2a_out = t_["a2a_in"], t_["a2a_out"]

    ctx = contextlib.ExitStack()
    with ctx:
        pers = ctx.enter_context(tc.tile_pool(name="pers", bufs=1))
        ident = pers.tile([128, 128], F32)
        mask_sb = pers.tile([128, 4, 512], BF16)
        cmasks.make_identity(nc, ident[:])
        nc.sync.dma_start(out=mask_sb[:], in_=masksd_d)

        loop_ctx = tc.For_i(0, repeat, 1) if repeat else contextlib.nullcontext()
        with loop_ctx:
            # ---- scope A: q/k/v + attention output, alive through phase 2 ----
            ctxA = contextlib.ExitStack()
            with ctxA:
                qkv = ctxA.enter_context(tc.tile_pool(name="qkv", bufs=1))
                q_sb = qkv.tile([128, 2 * H_LOC, T], F32R)      # (D, bh, T)
                k_sb = qkv.tile([128, 2 * H_LOC, T], F32R)
                v_sb = qkv.tile([128, 2 * H_LOC, T // 128, 132], BF16)
                yT_sb = qkv.tile([128, H_LOC, BT], F32R)
                nc.vector.memset(v_sb[:], 0.0)
                nc.vector.memset(v_sb[:, :, :, 128:129], 1.0)

                if 1 in phases:
                    _emit_qkv_rope(nc, tc, xT_d, wqk_d, wv_d, cosT_d, sinTs_d,
                                   q_sb, k_sb, v_sb)
                elif 2 in phases:
                    ztmp = qkv.tile([128, 512], F32)
                    nc.vector.memset(ztmp[:], 0.001)
                    for bh_ in range(2 * H_LOC):
                        for i_ in range(T // 512):
                            nc.vector.tensor_copy(out=q_sb[:, bh_, i_*512:(i_+1)*512], in_=ztmp[:])
                            nc.vector.tensor_copy(out=k_sb[:, bh_, i_*512:(i_+1)*512], in_=ztmp[:])
                if 2 in phases:
                    _emit_attention(nc, tc, q_sb, k_sb, v_sb, yT_sb, mask_sb, ident)
                if 3 in phases and 2 not in phases:
                    ztmp2 = qkv.tile([128, 512], F32)
                    nc.vector.memset(ztmp2[:], 0.001)
                    for hl_ in range(H_LOC):
                        for i_ in range(BT // 512):
                            nc.vector.tensor_copy(out=yT_sb[:, hl_, i_*512:(i_+1)*512], in_=ztmp2[:])
                if 3 in phases:
                    # spill head-sharded yT per batch; half hb block d =
                    # batch-hb tokens [256d, 256d+256) (ready after that
                    # batch's two heads finish attention)
                    for hb in range(2):
                        for hl in range(H_LOC):
                            for d in range(N_CORES):
                                nc.gpsimd.dma_start(
                                    out=a2a_in[hb][d, hl * 128:(hl + 1) * 128, :],
                                    in_=yT_sb[:, hl, hb * T + 256 * d:hb * T + 256 * (d + 1)])
                if not phases:
                    z = qkv.tile([128, 8], F32)
                    nc.vector.memset(z[:], 0.0)

            if 3 in phases:
                if use_collective:
                    for hb in range(2):
                        nc.gpsimd.collective_compute(
                            "AllToAll", mybir.AluOpType.bypass,
                            replica_groups=[list(range(N_CORES))],
                            ins=[a2a_in[hb].opt()], outs=[a2a_out[hb].opt()],
                        )
                    src = a2a_out
                else:
                    src = a2a_in
                _emit_proj(nc, tc, src, wproj_d, y_d)


def _emit_qkv_rope(nc, tc, xT_d, wqk_d, wv_d, cosT_d, sinTs_d, q_sb, k_sb, v_sb):
    ctx = contextlib.ExitStack()
    with ctx:
        p1 = ctx.enter_context(tc.tile_pool(name="p1", bufs=1))
        xp = ctx.enter_context(tc.tile_pool(name="xp", bufs=6))
        cp = ctx.enter_context(tc.tile_pool(name="cp", bufs=1))
        rp = ctx.enter_context(tc.tile_pool(name="rp", bufs=2))
        qps = ctx.enter_context(tc.tile_pool(name="qps", bufs=2, space="PSUM"))
        kps = ctx.enter_context(tc.tile_pool(name="kps", bufs=2, space="PSUM"))
        vps = ctx.enter_context(tc.tile_pool(name="vps", bufs=4, space="PSUM"))

        wqk_sb = p1.tile([128, N_CCH, 512], F32R)
        wv_sb = p1.tile([128, N_CCH, 256], F32R)
        nc.sync.dma_start(out=wqk_sb[:], in_=wqk_d.transpose([1, 0, 2]))
        nc.sync.dma_start(out=wv_sb[:], in_=wv_d.transpose([1, 0, 2]))

        for g in range(N_GRP):
            b = g // (T // 512)
            pos0 = 512 * (g % (T // 512))
            q_ps = [qps.tile([128, 512], F32, name="q_ps", tag="q_ps") for _ in range(2)]
            k_ps = [kps.tile([128, 512], F32, name="k_ps", tag="k_ps") for _ in range(2)]
            v_ps = [vps.tile([128, 512], F32, name="v_ps", tag="v_ps") for _ in range(4)]
            for c in range(N_CCH):
                xt = xp.tile([128, 512], F32R, name="xt", tag="xt")
                nc.sync.dma_start(out=xt[:], in_=xT_d[c * 128:(c + 1) * 128,
                                                      g * 512:(g + 1) * 512])
                st, sp = (c == 0), (c == N_CCH - 1)
                for h in range(2):
                    nc.tensor.matmul(q_ps[h][:], wqk_sb[:, c, h * 128:(h + 1) * 128],
                                     xt[:], start=st, stop=sp)
                    nc.tensor.matmul(k_ps[h][:], wqk_sb[:, c, 256 + h * 128:256 + (h + 1) * 128],
                                     xt[:], start=st, stop=sp)
                for ts in range(4):
                    nc.tensor.matmul(v_ps[ts][:, 0:256], xt[:, ts * 128:(ts + 1) * 128],
                                     wv_sb[:, c, :], start=st, stop=sp)
            cos_t = cp.tile([128, 512], F32, name="cos_t", tag="cos_t")
            sin_t = cp.tile([128, 512], F32, name="sin_t", tag="sin_t")
            nc.sync.dma_start(out=cos_t[:], in_=cosT_d[:, pos0:pos0 + 512])
            nc.sync.dma_start(out=sin_t[:], in_=sinTs_d[:, pos0:pos0 + 512])
            # PSUM-freeing drains first: q/k on DVE, v on ACT (idle in phase 1)
            for h in range(2):
                bh = b * 2 + h
                for ps, dst in ((q_ps[h], q_sb), (k_ps[h], k_sb)):
                    nc.vector.tensor_copy(out=dst[:, bh, pos0:pos0 + 512], in_=ps[:])
            for ts in range(4):
                kt = 4 * (g % (T // 512)) + ts
                for h in range(2):
                    bh = b * 2 + h
                    nc.scalar.copy(out=v_sb[:, bh, kt, 0:128],
                                   in_=v_ps[ts][:, h * 128:(h + 1) * 128])
            # RoPE (reads/writes q_sb/k_sb, PSUM already released)
            for h in range(2):
                bh = b * 2 + h
                for dst in (q_sb, k_sb):
                    dslc = dst[:, bh, pos0:pos0 + 512]
                    rot = rp.tile([128, 512], F32R, name="rot", tag="rot")
                    nc.gpsimd.dma_start(out=rot[0:64, :], in_=dslc[64:128, :])
                    nc.gpsimd.dma_start(out=rot[64:128, :], in_=dslc[0:64, :])
                    tsin = rp.tile([128, 512], F32, name="tsin", tag="tsin")
                    nc.vector.tensor_mul(out=tsin[:], in0=rot[:], in1=sin_t[:])
                    nc.vector.tensor_mul(out=dslc, in0=dslc, in1=cos_t[:])
                    nc.vector.tensor_add(out=dslc, in0=dslc, in1=tsin[:])


def _emit_attention(nc, tc, q_sb, k_sb, v_sb, yT_sb, mask_sb, ident):
    ctx = contextlib.ExitStack()
    with ctx:
        ptp = ctx.enter_context(tc.tile_pool(name="ptp", bufs=3))
        osb = ctx.enter_context(tc.tile_pool(name="osb", bufs=3))
        ops = ctx.enter_context(tc.tile_pool(name="ops", bufs=4, space="PSUM"))
        stps = ctx.enter_context(tc.tile_pool(name="stps", bufs=3, space="PSUM"))
        otps = ctx.enter_context(tc.tile_pool(name="otps", bufs=1, space="PSUM"))

        for bh in range(2 * H_LOC):
            b, hl = bh // 2, bh % 2
            for qt in range(T // 512):
                o_ps = [ops.tile([128, 512], F32, name="o_ps", tag="o_ps")
                        for _ in range(4)]
                nkt = 4 * qt + 4
                for kt in range(nkt):
                    st_ps = stps.tile([128, 512], F32, name="st_ps", tag="st_ps")
                    nc.tensor.matmul(st_ps[:], k_sb[:, bh, kt * 128:(kt + 1) * 128],
                                     q_sb[:, bh, qt * 512:(qt + 1) * 512],
                                     start=True, stop=True)
                    pt = ptp.tile([128, 512], BF16, name="pt", tag="pt")
                    nc.scalar.activation(out=pt[:], in_=st_ps[:],
                                         func=mybir.ActivationFunctionType.Exp,
                                         scale=float(SCALE))
                    jj = kt - 4 * qt
                    if jj >= 0:
                        nc.vector.tensor_mul(out=pt[:], in0=pt[:],
                                             in1=mask_sb[:, jj, :])
                    for qs in range(4):
                        nc.tensor.matmul(o_ps[qs][:, 0:129],
                                         pt[:, qs * 128:(qs + 1) * 128],
                                         v_sb[:, bh, kt, 0:129],
                                         start=(kt == 0), stop=(kt == nkt - 1))
                for qs in range(4):
                    recip = osb.tile([128, 1], F32, name="recip", tag="recip")
                    nc.vector.reciprocal(out=recip[:], in_=o_ps[qs][:, 128:129])
                    o_t = osb.tile([128, 128], F32, name="o_t", tag="o_t")
                    nc.vector.tensor_scalar_mul(out=o_t[:], in0=o_ps[qs][:, 0:128],
                                                scalar1=recip[:])
                    ot_ps = otps.tile([128, 512], F32, name="ot_ps", tag="ot_ps")[:, 0:128]
                    nc.tensor.transpose(ot_ps, o_t[:], ident[:])
                    tok0 = b * T + qt * 512 + qs * 128
                    nc.vector.tensor_copy(out=yT_sb[:, hl, tok0:tok0 + 128],
                                          in_=ot_ps)


def _emit_proj(nc, tc, src, wproj_d, y_d):
    ctx = contextlib.ExitStack()
    with ctx:
        wpr = ctx.enter_context(tc.tile_pool(name="wpr", bufs=1))
        yap = ctx.enter_context(tc.tile_pool(name="yap", bufs=1))
        outp = ctx.enter_context(tc.tile_pool(name="outp", bufs=3))
        pjps = ctx.enter_context(tc.tile_pool(name="pjps", bufs=8, space="PSUM"))

        # resident w_proj.T (131 KB/partition); streams in while the
        # collective runs (no data dependency between them)
        wp_sb = wpr.tile([128, N_CCH, C], F32R)
        for cc in range(N_CCH):
            nc.sync.dma_start(out=wp_sb[:, cc, :], in_=wproj_d[cc])

        ya_sb = yap.tile([128, 2, N_CCH, 256], F32R)
        for hb in range(2):
            for cc in range(N_CCH):
                nc.sync.dma_start(
                    out=ya_sb[:, hb, cc, :],
                    in_=src[hb][cc // 2, (cc % 2) * 128:((cc % 2) + 1) * 128, :])

        # y_d rows [0,256) = my batch-0 tokens, [256,512) = my batch-1 tokens
        for hb in range(2):
            for nf in range(C // 512):
                pj_ps = [pjps.tile([128, 512], F32, name="pj_ps", tag="pj_ps")
                         for _ in range(2)]
                for cc in range(N_CCH):
                    for mt in range(2):
                        nc.tensor.matmul(pj_ps[mt][:],
                                         ya_sb[:, hb, cc, mt * 128:(mt + 1) * 128],
                                         wp_sb[:, cc, nf * 512:(nf + 1) * 512],
                                         start=(cc == 0), stop=(cc == N_CCH - 1))
                for mt in range(2):
                    o_sb = outp.tile([128, 512], F32, name="o_sb", tag="o_sb")
                    nc.vector.tensor_copy(out=o_sb[:], in_=pj_ps[mt][:])
                    row0 = hb * 256 + mt * 128
                    nc.sync.dma_start(out=y_d[row0:row0 + 128,
                                              nf * 512:(nf + 1) * 512], in_=o_sb[:])


# ---------------- host side ----------------

def _rope_tables():
    inv_freq = 1.0 / (ROPE_BASE ** (np.arange(0, D, 2, dtype=np.float32) / D))
    ang = np.arange(T, dtype=np.float32)[:, None] * inv_freq[None, :]   # (T, D/2)
    cos = np.concatenate([np.cos(ang), np.cos(ang)], axis=-1).astype(np.float32)
    sin = np.concatenate([np.sin(ang), np.sin(ang)], axis=-1).astype(np.float32)
    cosT = np.ascontiguousarray(cos.T)                                  # (D, T)
    sinTs = np.ascontiguousarray(sin.T)
    sinTs[0:64, :] *= -1.0
    return cosT, sinTs


def _diag_masks():
    kp = np.arange(128)[:, None]
    qf = np.arange(512)[None, :]
    m = np.stack([(128 * jj + kp <= qf) for jj in range(4)], axis=1)
    return m.astype(ml_dtypes.bfloat16)                                 # (128, 4, 512)


def prep_in_maps(x, w_qkv, w_proj):
    x = np.asarray(x, dtype=np.float32)
    w_qkv = np.asarray(w_qkv, dtype=np.float32)
    w_proj = np.asarray(w_proj, dtype=np.float32)

    xT = np.ascontiguousarray(x.reshape(BT, C).T)                        # (C, BT)
    wprojT = np.ascontiguousarray(w_proj.T).reshape(N_CCH, 128, C)
    cosT, sinTs = _rope_tables()
    masksd = _diag_masks()

    in_maps = []
    for r in range(N_CORES):
        rows = slice(256 * r, 256 * (r + 1))
        wq = np.ascontiguousarray(w_qkv[0 * C:1 * C][rows].T).reshape(N_CCH, 128, 256)
        wk = np.ascontiguousarray(w_qkv[1 * C:2 * C][rows].T).reshape(N_CCH, 128, 256)
        wv = np.ascontiguousarray(w_qkv[2 * C:3 * C][rows].T).reshape(N_CCH, 128, 256)
        wqk = np.concatenate([wq, wk], axis=2)                           # (16,128,512)
        in_maps.append({
            "xT": xT, "wqk": np.ascontiguousarray(wqk), "wv": wv,
            "wproj": wprojT, "cosT": cosT, "sinTs": sinTs, "masksd": masksd,
        })
    return in_maps


def assemble(results):
    y0 = np.concatenate([results[r]["y"][0:256] for r in range(N_CORES)], axis=0)
    y1 = np.concatenate([results[r]["y"][256:512] for r in range(N_CORES)], axis=0)
    return np.stack([y0, y1], axis=0).reshape(B, T, C).astype(np.float32)


_CACHED_NC = None


def kernel(x, w_qkv, w_proj):
    global _CACHED_NC
    if _CACHED_NC is None:
        _CACHED_NC = build()
    in_maps = prep_in_maps(x, w_qkv, w_proj)
    res = run_bass_kernel_spmd(_CACHED_NC, in_maps, list(range(N_CORES)))
    return assemble(res.results)
